# revision 1
# baseline (speedup 1.0000x reference)
"""Trainium2 Bass kernel for nn_MultiHeadAttention_6055903887702.

Sharding: one attention head per NeuronCore (H == n_cores == 8). Each core
computes, for its head h:
    A_h  = Wq_h Wk_h^T  (host-precomputed, so Q/K projections collapse)
    GT_h = A_h^T X^T, V_h = X Wv_h                              (f32r matmuls)
    ST_h = X G^T      (scores, transposed layout [t, s])        (f32r matmuls)
    P_h  = exp(ST_h / sqrt(E))   (unnormalized, no max-sub — logits ~N(0,1))
    colsum[s] = sum_t P_h[t, s]  (DVE accumulate + one f32r ones-matmul for
                                  the cross-partition reduction)
    OT_h = V_h^T P_h / colsum    ([n, s])                       (fp16 matmuls)
    Z_h  = O_h Wp_h  (partial output [s, m])                    (fp16 matmuls)
The host passes x pre-transposed ([E, B*S]) so no on-device transposes are
needed anywhere; the partials are summed on the host and bp is added.
Projections read x / Wq / Wk / Wv as float32r (full fp32 bytes, PE rounds
internally, 1 cycle/row at N>=256); Q/K/V/P are stored fp16 in SBUF so both
K_b and V_b stay resident per batch. All PSUM accumulation is fp32.
"""

import numpy as np

import concourse.bacc as bacc
import concourse.mybir as mybir
import concourse.tile as tile
from concourse.bass import ds, ts
from concourse.bass_utils import run_bass_kernel_spmd
from concourse.masks import make_identity

H = 8
E = 768
B = 4
S = 2048
TOK = B * S          # 8192 tokens
P = 128              # partitions
EC = E // P          # 6 chunks of the embedding dim
SC = 512             # s-chunk (query block, one PSUM bank wide)
NSC = S // SC        # 4 s-chunks per batch
NT = S // P          # 16 key tiles per batch
VN = 384             # V / Z free-dim chunk (768 = 2 x 384, >=256 keeps f32r fast)

F32 = mybir.dt.float32
F32R = mybir.dt.float32r
F16 = mybir.dt.float16

_NC_CACHE = None


def _build_nc():
    nc = bacc.Bacc("TRN2", target_bir_lowering=False, debug=False, num_devices=H)

    xT = nc.dram_tensor("xT", [E, TOK], F32R, kind="ExternalInput")
    a = nc.dram_tensor("a", [E, E], F32R, kind="ExternalInput")
    wv = nc.dram_tensor("wv", [E, E], F32R, kind="ExternalInput")
    out = nc.dram_tensor("out", [TOK, E], F32, kind="ExternalOutput")

    xT3 = xT[:].rearrange("(eo ei) t -> ei eo t", ei=P)
    a3 = a[:].rearrange("(eo ei) f -> ei eo f", ei=P)
    wv3 = wv[:].rearrange("(eo ei) d -> ei eo d", ei=P)

    inv_sqrt_e = float(1.0 / np.sqrt(E))

    with tile.TileContext(nc) as tc:
        with (
            tc.tile_pool(name="wpool", bufs=1) as wpool,
            tc.tile_pool(name="kvpool", bufs=1) as kvpool,
            tc.tile_pool(name="work", bufs=2) as work,
            tc.tile_pool(name="pexps", bufs=18) as pexps,
            tc.tile_pool(name="zs", bufs=3) as zs,
            tc.tile_pool(name="ps_proj", bufs=3, space="PSUM") as ps_proj,
            tc.tile_pool(name="ps_sc", bufs=2, space="PSUM") as ps_sc,
            tc.tile_pool(name="ps_cs", bufs=1, space="PSUM") as ps_cs,
            tc.tile_pool(name="ps_ot", bufs=2, space="PSUM") as ps_ot,
        ):
            a_sb = wpool.tile([P, EC, E], F32R, name="a_sb")
            wv_sb = wpool.tile([P, EC, E], F32R, name="wv_sb")
            # DMA issue order: first x chunk + wv first half gate the first
            # V-proj group; a/wp are deferred to phase 2.
            xtb = {}
            xtb[(0, 0)] = work.tile([P, EC, SC], F32R, tag="xtb", bufs=5,
                                    name="xtb_0_0")
            nc.sync.dma_start(xtb[(0, 0)][:], xT3[:, :, ds(0, SC)])
            for nch in range(E // VN):
                nc.sync.dma_start(
                    wv_sb[:, :, ds(nch * VN, VN)], wv3[:, :, ds(nch * VN, VN)]
                )
            ident = wpool.tile([P, P], F32, name="ident")
            make_identity(nc, ident[:])
            ones_f32 = wpool.tile([P, P], F32, name="ones_f32")
            nc.vector.memset(ones_f32[:], 1.0)
            ones = wpool.tile([P, P], F32R, name="ones")
            nc.vector.tensor_copy(out=ones[:], in_=ones_f32[:])

            # Warm the PE (HAM clock ramp) with throwaway matmuls while the
            # first weight/x DMAs are in flight, so real matmuls start at the
            # full 2.4 GHz rate.
            for w in range(26):
                pw = ps_cs.tile([P, P], F32, tag="ps_cs", name="pw")
                nc.tensor.matmul(pw[:], ones[:], ones[:], start=True, stop=True)

            for b in range(B):
                tok0 = b * S
                v = kvpool.tile([P, NT, E], F16, tag="v", name=f"v_{b}")

                # ---- phase 1: V_b (x chunks stay resident for scores) ----
                for tci in range(NSC):
                    if (b, tci) not in xtb:
                        xtb[(b, tci)] = work.tile(
                            [P, EC, SC], F32R, tag="xtb", bufs=5,
                            name=f"xtb_{b}_{tci}"
                        )
                        nc.sync.dma_start(
                            xtb[(b, tci)][:], xT3[:, :, ds(tok0 + tci * SC, SC)]
                        )
                    xts = xtb[(b, tci)]
                    # nch outer: consumes wv's first half before the second
                    # arrives at startup
                    for nch in range(E // VN):
                        for tt in range(SC // P):
                            t_tile = tci * (SC // P) + tt
                            pv = ps_proj.tile([P, VN], F32, tag="ps_proj", name="pv")
                            for e in range(EC):
                                nc.tensor.matmul(
                                    pv[:],
                                    xts[:, e, ts(tt, P)],
                                    wv_sb[:, e, ds(nch * VN, VN)],
                                    start=(e == 0),
                                    stop=(e == EC - 1),
                                )
                            nc.vector.tensor_copy(
                                out=v[:, t_tile, ds(nch * VN, VN)], in_=pv[:]
                            )

                # ---- phase 2: attention per s-chunk ----
                for sci in range(NSC):
                    s0 = tok0 + sci * SC
                    if b == 0 and sci == 0:
                        # deferred weight load: needed from here on
                        nc.sync.dma_start(a_sb[:], a3)
                    # G^T = A^T X^T: the query-side operand; x slice is the
                    # s-chunk of the resident batch chunks (s range == t range)
                    gt = work.tile([P, EC, SC], F32R, tag="gt", name=f"gt_{b}_{sci}")
                    for f in range(EC):
                        pq = ps_proj.tile([P, SC], F32, tag="ps_proj", name="pq")
                        for e in range(EC):
                            nc.tensor.matmul(
                                pq[:],
                                a_sb[:, e, ts(f, P)],
                                xtb[(b, sci)][:, e, :],
                                start=(e == 0),
                                stop=(e == EC - 1),
                            )
                        nc.vector.tensor_copy(out=gt[:, f, :], in_=pq[:])

                    # scores + exp; partial column sums accumulate on DVE in
                    # f32r; one f32r ones-matmul then reduces across
                    # partitions (replaces 16 PE colsum matmuls per s-chunk)
                    csum = work.tile([P, SC], F32R, tag="csum", name="csum", bufs=1)
                    pexp_tiles = []
                    for t in range(NT):
                        pst = ps_sc.tile([P, SC], F32, tag="ps_sc", name="pst")
                        for f in range(EC):
                            nc.tensor.matmul(
                                pst[:],
                                xtb[(b, t // 4)][:, f, ts(t % 4, P)],
                                gt[:, f, :],
                                start=(f == 0),
                                stop=(f == EC - 1),
                            )
                        pe_t = pexps.tile([P, SC], F16, tag="pexp", name=f"pexp_{t}")
                        nc.scalar.activation(
                            pe_t[:],
                            pst[:],
                            mybir.ActivationFunctionType.Exp,
                            scale=inv_sqrt_e,
                        )
                        pexp_tiles.append(pe_t)
                        if t == 0:
                            nc.vector.tensor_copy(out=csum[:], in_=pe_t[:])
                        else:
                            nc.vector.tensor_add(
                                out=csum[:], in0=csum[:], in1=pe_t[:]
                            )
                    pcs = ps_cs.tile([P, SC], F32, tag="ps_cs", name="pcs")
                    nc.tensor.matmul(
                        pcs[:], ones[:], csum[:], start=True, stop=True
                    )
                    rec = work.tile([P, SC], F32, tag="rec", name="rec", bufs=2)
                    nc.vector.reciprocal(rec[:], pcs[:])

                    # Z = P-hat^T U directly (U = X Wv Wp resident as `v`;
                    # O is never materialized)
                    rec_col = work.tile([P, NSC], F32, tag="rec_col",
                                        name="rec_col", bufs=2)
                    first = True
                    for st in range(SC // P):
                        for mch in range(E // VN):
                            pz = ps_ot.tile([P, VN], F32, tag="ps_ot", name="pz")
                            for t in range(NT):
                                nc.tensor.matmul(
                                    pz[:],
                                    pexp_tiles[t][:, ts(st, P)],
                                    v[:, t, ds(mch * VN, VN)],
                                    start=(t == 0),
                                    stop=(t == NT - 1),
                                )
                            if first:
                                # bring 1/colsum to per-partition layout via PE
                                # transposes (off the critical z-mult path)
                                first = False
                                for st2 in range(SC // P):
                                    tp = ps_cs.tile([P, P], F32, tag="ps_cs",
                                                    name="tp")
                                    nc.tensor.transpose(
                                        tp[:], rec[:, ts(st2, P)], ident[:]
                                    )
                                    nc.vector.tensor_copy(
                                        out=rec_col[:, st2 : st2 + 1],
                                        in_=tp[:, 0:1],
                                    )
                            z = zs.tile([P, VN], F32, tag="z", name="z")
                            nc.vector.tensor_scalar_mul(
                                z[:], pz[:], rec_col[:, st : st + 1]
                            )
                            nc.sync.dma_start(
                                out[ds(s0 + st * P, P), ds(mch * VN, VN)], z[:]
                            )

    nc.compile()
    return nc


def get_nc():
    global _NC_CACHE
    if _NC_CACHE is None:
        _NC_CACHE = _build_nc()
    return _NC_CACHE


def make_in_maps(x, Wq, Wk, Wv, Wp):
    x = np.asarray(x, dtype=np.float32)
    Wq = np.asarray(Wq, dtype=np.float32)
    Wk = np.asarray(Wk, dtype=np.float32)
    Wv = np.asarray(Wv, dtype=np.float32)
    Wp = np.asarray(Wp, dtype=np.float32)
    xT = np.ascontiguousarray(x.reshape(TOK, E).T)
    in_maps = []
    for h in range(H):
        # A_h[e, f] = sum_d Wq_h[e, d] Wk_h[f, d]: collapses the Q and K
        # projections into one on-device G = X @ A projection.
        a_h = np.ascontiguousarray(Wq[h] @ Wk[h].T)
        # C_h = Wv_h @ Wp_h folds the value and output projections: the
        # device computes U = X @ C_h once and Z = P_hat^T U directly.
        c_h = np.ascontiguousarray(Wv[h] @ Wp[h * E : (h + 1) * E])
        in_maps.append(
            {
                "xT": xT,
                "a": a_h,
                "wv": c_h,
            }
        )
    return in_maps


def kernel(x, Wq, Wk, Wv, Wp, bp):
    nc = get_nc()
    in_maps = make_in_maps(x, Wq, Wk, Wv, Wp)
    res = run_bass_kernel_spmd(nc, in_maps, core_ids=list(range(H)))
    acc = res.results[0]["out"].copy()
    for h in range(1, H):
        acc += res.results[h]["out"]
    acc += np.asarray(bp, dtype=np.float32)
    return acc.reshape(B, S, E)



# revision 2
# speedup vs baseline: 1.0017x; 1.0017x over previous
"""Trainium2 Bass kernel for nn_MultiHeadAttention_6055903887702.

Sharding: one attention head per NeuronCore (H == n_cores == 8). The host
folds the projections (A_h = Wq_h Wk_h^T collapses Q/K; C_h = Wv_h Wp_h
collapses V/proj) and quantizes every matmul operand into an fp8e4 (e4m3)
hi/lo pair: x ~ x8 + xr with xr = e4m3(x - x8). Every matmul on the device
runs in fp8 DoubleRow mode (2 K-chunks per instruction, 0.5 cycles/row) and
keeps the three significant products hi*hi + hi*lo + lo*hi, dropping only
the lo*lo term (~0.13%). That is 3 fp8-DR passes = 0.75x the f32r cycle
count of the same contraction while keeping absmax-rel error ~3e-3.

Per head h on its core (all accumulation in f32 PSUM):
    U' = X C'_h        [t, m]   (C' = C * 32; columns 768.. are a constant
                                 ones block so Z's matmul emits the softmax
                                 colsum for free)
    G' = X A'_h        [e', s]  (A' = A * 16)
    S  = X^T G'        [t, s]   logits * 16 * sqrt(E)
    P~ = exp(S / (16 sqrt(E)) - ln 8)
    Z  = P~^T U'       [s, m | colsum]
    z  = Z / colsum    (DVE reciprocal + per-partition scalar multiply)
Host: out = (sum_h z_h) / 32 + bp. The exp's bias -ln 8 keeps p in e4m3
range (max logit ~6.7 -> p~ <= 101 < 240); the 1/8 cancels in Z/colsum.
Output is written f16 (error contribution ~5e-4 of a partial, halves the
output DMA); the host sums the 8 partials in f32.

Every quantized intermediate (P, G, U hi/lo) is stored in per-DoubleRow-pair
tiles ([128, 2, width]) rather than one big tile: the tile framework tracks
dependencies per tile, and a single shared tile serializes the producing and
consuming engines into a write-after-read ping-pong that paces the whole
scores phase at the Act+GpSimd chain rate instead of the PE rate. Drains are
strictly one-directional pipelines:
    scores: Act exp->pf16 pair; DVE cast pf->p8 pair; Pool sub -> pr pair
    G/U:    Act cast psum->stage; DVE sub(psum,stage)->lo; Pool copy
            stage->hi
so no engine ever waits on a later engine in the chain.
"""

import numpy as np
import ml_dtypes

import concourse.bacc as bacc
import concourse.mybir as mybir
import concourse.tile as tile
from concourse.bass import ds, ts
from concourse.bass_utils import run_bass_kernel_spmd

H = 8
E = 768
B = 4
S = 2048
TOK = B * S
P = 128
EC = E // P          # 6 e-chunks
EP = EC // 2         # 3 e-chunk pairs (DoubleRow contracts 2 chunks/instr)
SC = 512             # s-chunk (query block)
NSC = S // SC        # 4 s-chunks per batch
NT = S // P          # 16 t-tiles per batch
TP = NT // 2         # 8 t-tile pairs
MCH = 384            # m chunk for U/Z (768 = 2 x 384)
CSW = 2              # width of the ones/colsum block appended to U
UW = E + CSW
SA = 16.0            # host scale on A  (keeps A / its residual normal-range)
SCC = 32.0           # host scale on C
ISC = float(1.0 / (SA * np.sqrt(E)))
PBIAS = float(-np.log(8.0))
NWARM = 28

E4NP = ml_dtypes.float8_e4m3
F32 = mybir.dt.float32
F16 = mybir.dt.float16
FP8 = mybir.dt.float8e4
DR = mybir.MatmulPerfMode.DoubleRow
EXP = mybir.ActivationFunctionType.Exp
CPY = mybir.ActivationFunctionType.Copy

_NC_CACHE = None


def _build_nc():
    nc = bacc.Bacc("TRN2", target_bir_lowering=False, debug=False, num_devices=H)

    x8d = nc.dram_tensor("x8", [P, EC, TOK], FP8, kind="ExternalInput")
    xrd = nc.dram_tensor("xr", [P, EC, TOK], FP8, kind="ExternalInput")
    a8d = nc.dram_tensor("a8", [P, EC, E], FP8, kind="ExternalInput")
    ard = nc.dram_tensor("ar", [P, EC, E], FP8, kind="ExternalInput")
    c8d = nc.dram_tensor("c8", [P, EC, E], FP8, kind="ExternalInput")
    crd = nc.dram_tensor("cr", [P, EC, E], FP8, kind="ExternalInput")
    outd = nc.dram_tensor("out", [TOK, E], F16, kind="ExternalOutput")

    mm = nc.tensor.matmul

    with tile.TileContext(nc) as tc:
        with (
            tc.tile_pool(name="wpool", bufs=1) as wpool,
            tc.tile_pool(name="xpool", bufs=2) as xpool,
            tc.tile_pool(name="upool", bufs=2) as upool,
            tc.tile_pool(name="gpool", bufs=2) as gpool,
            tc.tile_pool(name="ppool", bufs=2) as ppool,
            tc.tile_pool(name="spool", bufs=3) as spool,
            tc.tile_pool(name="stpool", bufs=4) as stpool,
            tc.tile_pool(name="zpool", bufs=3) as zpool,
            tc.tile_pool(name="rpool", bufs=2) as rpool,
            tc.tile_pool(name="ps_sc", bufs=2, space="PSUM") as ps_sc,
            tc.tile_pool(name="ps_z", bufs=2, space="PSUM") as ps_z,
            tc.tile_pool(name="ps_pj", bufs=4, space="PSUM") as ps_pj,
        ):
            ones = wpool.tile([P, 2, 256], FP8, name="ones")
            nc.gpsimd.memset(ones[:], 1.0)
            nbias = wpool.tile([P, 1], F32, name="nbias")
            nc.vector.memset(nbias[:], PBIAS)

            # PE p-state warmup while the first DMAs land
            for i in range(NWARM):
                pw = ps_sc.tile([P, SC], F32, tag="sc", name=f"warm{i}")
                mm(pw[:, 0:256], ones[:, :, 0:128], ones[:],
                   start=True, stop=True, perf_mode=DR)

            # weights: C first (U-proj of batch 0 gates the pipeline)
            c8 = wpool.tile([P, EC, E], FP8, name="c8")
            cr = wpool.tile([P, EC, E], FP8, name="cr")
            a8 = wpool.tile([P, EC, E], FP8, name="a8")
            ar = wpool.tile([P, EC, E], FP8, name="ar")

            xts = {}

            def load_x(b, interleave_with=None):
                t8 = xpool.tile([P, EC, S], FP8, tag="x8", name=f"x8_{b}")
                tr = xpool.tile([P, EC, S], FP8, tag="xr", name=f"xr_{b}")
                for ep in range(EP):
                    nc.sync.dma_start(t8[:, ds(2 * ep, 2), :],
                                      x8d[:, ds(2 * ep, 2), ds(b * S, S)])
                    nc.sync.dma_start(tr[:, ds(2 * ep, 2), :],
                                      xrd[:, ds(2 * ep, 2), ds(b * S, S)])
                    if interleave_with is not None:
                        wt, wd = interleave_with[ep]
                        nc.sync.dma_start(wt, wd)
                xts[b] = (t8, tr)

            # two-wave startup: weights first, then x(b0) in t-halves so the
            # first U-proj tiles are fed as early as possible
            for ep in range(EP):
                nc.sync.dma_start(c8[:, ds(2 * ep, 2), :],
                                  c8d[:, ds(2 * ep, 2), :])
            for ep in range(EP):
                nc.sync.dma_start(cr[:, ds(2 * ep, 2), :],
                                  crd[:, ds(2 * ep, 2), :])
            t8_0 = xpool.tile([P, EC, S], FP8, tag="x8", name="x8_0")
            tr_0 = xpool.tile([P, EC, S], FP8, tag="xr", name="xr_0")
            HT = S // 2
            for th in range(2):
                for ep in range(EP):
                    nc.sync.dma_start(
                        t8_0[:, ds(2 * ep, 2), ds(th * HT, HT)],
                        x8d[:, ds(2 * ep, 2), ds(th * HT, HT)])
                for ep in range(EP):
                    nc.sync.dma_start(
                        tr_0[:, ds(2 * ep, 2), ds(th * HT, HT)],
                        xrd[:, ds(2 * ep, 2), ds(th * HT, HT)])
            xts[0] = (t8_0, tr_0)
            for ep in range(EP):
                nc.sync.dma_start(a8[:, ds(2 * ep, 2), :],
                                  a8d[:, ds(2 * ep, 2), :])
            for ep in range(EP):
                nc.sync.dma_start(ar[:, ds(2 * ep, 2), :],
                                  ard[:, ds(2 * ep, 2), :])

            uts = {}

            def u_alloc(b):
                u8p = [upool.tile([P, 2, UW], FP8, tag=f"u8_{q}",
                                  name=f"u8_{b}_{q}") for q in range(TP)]
                urp = [upool.tile([P, 2, UW], FP8, tag=f"ur_{q}",
                                  name=f"ur_{b}_{q}") for q in range(TP)]
                for q in range(TP):
                    nc.vector.memset(u8p[q][:, :, ds(E, CSW)], 1.0)
                    nc.vector.memset(urp[q][:, :, ds(E, CSW)], 0.0)
                uts[b] = (u8p, urp)

            def u_proj(b, tts):
                x8b, xrb = xts[b]
                u8p, urp = uts[b]
                for tt in tts:
                    q, r = tt // 2, tt % 2
                    pus = []
                    for mc in range(2):
                        pu = ps_pj.tile([P, SC], F32, tag="pj",
                                        name=f"pu_{b}_{tt}_{mc}")
                        for ep in range(EP):
                            xw8 = x8b[:, ds(2 * ep, 2), ds(tt * P, P)]
                            xwr = xrb[:, ds(2 * ep, 2), ds(tt * P, P)]
                            m8 = c8[:, ds(2 * ep, 2), ds(mc * MCH, MCH)]
                            mr = cr[:, ds(2 * ep, 2), ds(mc * MCH, MCH)]
                            mm(pu[:, 0:MCH], xw8, m8, start=(ep == 0),
                               stop=False, perf_mode=DR)
                            mm(pu[:, 0:MCH], xw8, mr, start=False, stop=False,
                               perf_mode=DR)
                            mm(pu[:, 0:MCH], xwr, m8, start=False,
                               stop=(ep == EP - 1), perf_mode=DR)
                        pus.append(pu)
                    # one-directional drain: Act -> stage, DVE sub, Pool copy
                    sts = []
                    for mc in range(2):
                        ustg = stpool.tile([P, MCH], FP8, tag="stg",
                                           name=f"us_{b}_{tt}_{mc}")
                        nc.scalar.activation(ustg[:], pus[mc][:, 0:MCH], CPY)
                        sts.append(ustg)
                    for mc in range(2):
                        nc.vector.tensor_sub(
                            out=urp[q][:, r, ds(mc * MCH, MCH)],
                            in0=pus[mc][:, 0:MCH], in1=sts[mc][:])
                    for mc in range(2):
                        nc.gpsimd.tensor_copy(
                            out=u8p[q][:, r, ds(mc * MCH, MCH)],
                            in_=sts[mc][:])

            def g_proj(b, sci):
                x8b, xrb = xts[b]
                g8p = [gpool.tile([P, 2, SC], FP8, tag=f"g8_{i}",
                                  name=f"g8_{b}_{sci}_{i}") for i in range(EP)]
                grp = [gpool.tile([P, 2, SC], FP8, tag=f"gr_{i}",
                                  name=f"gr_{b}_{sci}_{i}") for i in range(EP)]
                for f in range(EC):
                    pg = ps_pj.tile([P, SC], F32, tag="pj",
                                    name=f"pg_{b}_{sci}_{f}")
                    for ep in range(EP):
                        aw8 = a8[:, ds(2 * ep, 2), ds(f * P, P)]
                        awr = ar[:, ds(2 * ep, 2), ds(f * P, P)]
                        xm8 = x8b[:, ds(2 * ep, 2), ds(sci * SC, SC)]
                        xmr = xrb[:, ds(2 * ep, 2), ds(sci * SC, SC)]
                        mm(pg[:], aw8, xm8, start=(ep == 0), stop=False,
                           perf_mode=DR)
                        mm(pg[:], aw8, xmr, start=False, stop=False,
                           perf_mode=DR)
                        mm(pg[:], awr, xm8, start=False, stop=(ep == EP - 1),
                           perf_mode=DR)
                    gstg = stpool.tile([P, SC], FP8, tag="stg",
                                       name=f"gs_{b}_{sci}_{f}")
                    nc.scalar.activation(gstg[:], pg[:], CPY)
                    nc.vector.tensor_sub(out=grp[f // 2][:, f % 2, :],
                                         in0=pg[:], in1=gstg[:])
                    nc.gpsimd.tensor_copy(out=g8p[f // 2][:, f % 2, :],
                                          in_=gstg[:])
                return g8p, grp

            def scores(b, sci, g8p, grp, p8p, prp):
                x8b, xrb = xts[b]
                for q in range(TP):
                    pf = spool.tile([P, 2, SC], F16, tag="pf",
                                    name=f"pf_{b}_{sci}_{q}")
                    for r in range(2):
                        tt = 2 * q + r
                        pst = ps_sc.tile([P, SC], F32, tag="sc",
                                         name=f"pst_{b}_{sci}_{tt}")
                        for ep in range(EP):
                            xw8 = x8b[:, ds(2 * ep, 2), ds(tt * P, P)]
                            xwr = xrb[:, ds(2 * ep, 2), ds(tt * P, P)]
                            mm(pst[:], xw8, g8p[ep][:], start=(ep == 0),
                               stop=False, perf_mode=DR)
                            mm(pst[:], xw8, grp[ep][:], start=False,
                               stop=False, perf_mode=DR)
                            mm(pst[:], xwr, g8p[ep][:], start=False,
                               stop=(ep == EP - 1), perf_mode=DR)
                        nc.scalar.activation(pf[:, r, :], pst[:], EXP,
                                             bias=nbias[:], scale=ISC)
                    for r in range(2):
                        nc.vector.tensor_copy(out=p8p[q][:, r, :],
                                              in_=pf[:, r, :])
                    for r in range(2):
                        nc.gpsimd.tensor_sub(out=prp[q][:, r, :],
                                             in0=pf[:, r, :],
                                             in1=p8p[q][:, r, :])

            def z_phase(b, sci, p8p, prp):
                u8p, urp = uts[b]
                rec = rpool.tile([P, NSC], F32, tag="rec", name=f"rec_{b}_{sci}")
                for st in range(NSC):
                    # mc=1 first: its trailing CSW ones-columns carry the
                    # colsum that normalizes both m-chunks of this st
                    for mc in (1, 0):
                        w = MCH + CSW if mc == 1 else MCH
                        pz = ps_z.tile([P, MCH + CSW], F32, tag="z",
                                       name=f"pz_{b}_{sci}_{st}_{mc}")
                        for kp in range(TP):
                            pw8 = p8p[kp][:, :, ds(st * P, P)]
                            pwr = prp[kp][:, :, ds(st * P, P)]
                            um8 = u8p[kp][:, :, ds(mc * MCH, w)]
                            umr = urp[kp][:, :, ds(mc * MCH, w)]
                            mm(pz[:, 0:w], pw8, um8, start=(kp == 0),
                               stop=False, perf_mode=DR)
                            mm(pz[:, 0:w], pw8, umr, start=False, stop=False,
                               perf_mode=DR)
                            mm(pz[:, 0:w], pwr, um8, start=False,
                               stop=(kp == TP - 1), perf_mode=DR)
                        if mc == 1:
                            nc.vector.reciprocal(rec[:, st:st + 1],
                                                 pz[:, MCH:MCH + 1])
                        z = zpool.tile([P, MCH], F16, tag="z",
                                       name=f"z_{b}_{sci}_{st}_{mc}")
                        nc.vector.tensor_scalar_mul(z[:], pz[:, 0:MCH],
                                                    rec[:, st:st + 1])
                        row = b * S + sci * SC + st * P
                        half = MCH // 2
                        nc.sync.dma_start(
                            outd[ds(row, P), ds(mc * MCH, half)],
                            z[:, 0:half])
                        nc.sync.dma_start(
                            outd[ds(row, P), ds(mc * MCH + half, half)],
                            z[:, ds(half, half)])

            def p_alloc(b, sci):
                p8p = [ppool.tile([P, 2, SC], FP8, tag=f"p8_{q}",
                                  name=f"p8_{b}_{sci}_{q}") for q in range(TP)]
                prp = [ppool.tile([P, 2, SC], FP8, tag=f"pr_{q}",
                                  name=f"pr_{b}_{sci}_{q}") for q in range(TP)]
                return p8p, prp

            u_alloc(0)
            u_proj(0, range(NT))
            g_cur = g_proj(0, 0)
            for b in range(B):
                for sci in range(NSC):
                    if sci == 1 and b + 1 < B:
                        load_x(b + 1)
                    p8p, prp = p_alloc(b, sci)
                    scores(b, sci, g_cur[0], g_cur[1], p8p, prp)
                    # fill PE while Act/DVE/Pool finish the last P tiles
                    if sci < NSC - 1:
                        g_next = g_proj(b, sci + 1)
                    elif b + 1 < B:
                        g_next = g_proj(b + 1, 0)
                    else:
                        g_next = None
                    z_phase(b, sci, p8p, prp)
                    if b + 1 < B and sci >= 2:
                        if sci == 2:
                            u_alloc(b + 1)
                            u_proj(b + 1, range(0, TP))
                        else:
                            u_proj(b + 1, range(TP, NT))
                    g_cur = g_next

    nc.compile()
    return nc


def get_nc():
    global _NC_CACHE
    if _NC_CACHE is None:
        _NC_CACHE = _build_nc()
    return _NC_CACHE


def _rearr(m):
    """[E, cols] -> [P, EC, cols] with e = eo*P + ei -> [ei][eo][col]."""
    cols = m.shape[1]
    return np.ascontiguousarray(
        m.reshape(EC, P, cols).transpose(1, 0, 2))


def _qpair(m):
    hi = m.astype(E4NP)
    lo = (m - hi.astype(np.float32)).astype(E4NP)
    return hi, lo


def make_in_maps(x, Wq, Wk, Wv, Wp):
    x = np.asarray(x, dtype=np.float32)
    Wq = np.asarray(Wq, dtype=np.float32)
    Wk = np.asarray(Wk, dtype=np.float32)
    Wv = np.asarray(Wv, dtype=np.float32)
    Wp = np.asarray(Wp, dtype=np.float32)
    xT = _rearr(np.ascontiguousarray(x.reshape(TOK, E).T))
    x8, xr = _qpair(xT)
    in_maps = []
    for h in range(H):
        a = _rearr(np.ascontiguousarray(Wq[h] @ Wk[h].T) * SA)
        c = _rearr(np.ascontiguousarray(
            Wv[h] @ Wp[h * E:(h + 1) * E]) * SCC)
        a8, ar = _qpair(a)
        c8, cr = _qpair(c)
        in_maps.append({"x8": x8, "xr": xr, "a8": a8, "ar": ar,
                        "c8": c8, "cr": cr})
    return in_maps


def kernel(x, Wq, Wk, Wv, Wp, bp):
    nc = get_nc()
    in_maps = make_in_maps(x, Wq, Wk, Wv, Wp)
    res = run_bass_kernel_spmd(nc, in_maps, core_ids=list(range(H)))
    acc = res.results[0]["out"].astype(np.float32)
    for h in range(1, H):
        acc += res.results[h]["out"].astype(np.float32)
    acc *= np.float32(1.0 / SCC)
    acc += np.asarray(bp, dtype=np.float32)
    return acc.reshape(B, S, E)
